# revision 5
# baseline (speedup 1.0000x reference)
"""Banded (Luong) attention TRN2 Bass kernel, 8-core SPMD.

Problem: h [4, 4096, 1024] f32, W [1024, 1024] f32, T_hist=256.
  K = h @ W.T ; scores = (h @ K^T) / sqrt(H) with causal band
  (q - 255 <= k <= q); out = softmax(scores) @ h.

Sharding: data-parallel over batch (4) x sequence halves (2) -> 8 cores,
no cross-core communication. Each core handles 2048 queries; its key
region is 18 blocks of 128 (2 lead blocks for the band history,
zero-padded for the first half of each sequence).

Per-core algorithm (all matmuls in float32r = fp32 with 11-bit mantissa,
full PE speed):
  Q' = h_q @ W            (query-side projection; scores = Q' @ h_k^T)
  ST[k, q] = h_k @ Q'^T   computed per 256-query pair over 4 key blocks,
                          masked additively, exp with fused 1/32 scale
  ctx[q, :] = PT^T @ h    accumulated over 3 key blocks; row sums via a
                          ones-column matmul; normalize during the
                          PSUM->SBUF copy with a per-partition 1/sum.
"""

import numpy as np

import concourse.bass as bass
import concourse.mybir as mybir
import concourse.tile as tile
from concourse import bacc

B, T, H = 4, 4096, 1024
T_HIST = 256
N_CORES = 8
QB = 16            # 128-row query blocks per core
RB = QB + 2        # key-region blocks per core (2 lead blocks)
QPC = 2048         # queries per core
F32R = mybir.dt.float32r
F32 = mybir.dt.float32
NEG = np.float32(-1e9)
INV_SQRT_H = 1.0 / 32.0

_CACHE = {}


def _kernel_body(tc, out, hTr, haug, Wr, cmask, bmask, onesd):
    nc = tc.nc

    with (
        tc.tile_pool(name="singles", bufs=1) as singles,
        tc.tile_pool(name="haug", bufs=1) as haug_pool,
        tc.tile_pool(name="qt", bufs=1) as qt_pool,
        tc.tile_pool(name="pt", bufs=2) as pt_pool,
        tc.tile_pool(name="ctxs", bufs=3) as ctxs_pool,
        tc.tile_pool(name="recip", bufs=4) as recip_pool,
        tc.tile_pool(name="qtps", bufs=2, space="PSUM") as qtps_pool,
        tc.tile_pool(name="st", bufs=2, space="PSUM") as st_pool,
        tc.tile_pool(name="ctx", bufs=1, space="PSUM") as ctx_pool,
    ):
        # --- resident inputs ---
        hT_sb = singles.tile([128, 8, RB * 128], F32R)   # 9.44 MiB
        nc.sync.dma_start(hT_sb[:], hTr[:])
        W_sb = singles.tile([128, 8, H], F32R)           # 4 MiB
        nc.sync.dma_start(W_sb[:], Wr[:])
        cm_sb = singles.tile([128, 4, 256], F32)
        nc.sync.dma_start(cm_sb[:], cmask[:])
        bm_sb = singles.tile([128, 2, 256], F32)
        nc.sync.dma_start(bm_sb[:], bmask[:])
        ones_sb = singles.tile([128, 2], F32R)
        nc.sync.dma_start(ones_sb[:], onesd[:])

        for c in range(2):  # two chunks of 8 query blocks
            rb0 = c * 8                      # first region block of chunk
            # values for this chunk: region blocks rb0..rb0+9, keys-major
            ha = haug_pool.tile([128, 10, H], F32R)      # 5.24 MiB
            nc.sync.dma_start(
                ha[:], haug[rb0 : rb0 + 10].rearrange("b p f -> p b f")
            )

            # --- projection: Q'T[m, q] for this chunk's 1024 queries ---
            qt = qt_pool.tile([128, 8, 1024], F32R)      # 4 MiB
            qcol0 = 256 + c * 1024           # region col of chunk's queries
            for tt in range(2):
                for mc in range(8):
                    ps = qtps_pool.tile([128, 512], F32, tag="qtps")
                    for oc in range(8):
                        nc.tensor.matmul(
                            ps[:],
                            W_sb[:, oc, mc * 128 : (mc + 1) * 128],
                            hT_sb[:, oc, qcol0 + tt * 512 : qcol0 + (tt + 1) * 512],
                            start=(oc == 0),
                            stop=(oc == 7),
                        )
                    nc.vector.tensor_copy(
                        qt[:, mc, tt * 512 : (tt + 1) * 512], ps[:]
                    )

            # --- attention: 4 pairs (of 2 query blocks) per chunk ---
            for p in range(4):
                P = c * 4 + p                # global pair index
                st = st_pool.tile([128, 4, 256], F32)
                for r in range(4):
                    for mc in range(8):
                        nc.tensor.matmul(
                            st[:, r, :],
                            hT_sb[:, mc, (2 * P + r) * 128 : (2 * P + r + 1) * 128],
                            qt[:, mc, p * 256 : (p + 1) * 256],
                            start=(mc == 0),
                            stop=(mc == 7),
                        )
                if P == 0:
                    nc.vector.tensor_add(st[:, 0:2, :], st[:, 0:2, :], bm_sb[:])
                    nc.vector.tensor_add(
                        st[:, 2:4, :], st[:, 2:4, :], cm_sb[:, 2:4, :]
                    )
                else:
                    nc.vector.tensor_add(st[:], st[:], cm_sb[:])
                pt = pt_pool.tile([128, 4, 256], F32R)
                nc.scalar.activation(
                    pt[:], st[:], mybir.ActivationFunctionType.Exp,
                    scale=INV_SQRT_H,
                )

                for e in range(2):           # the two query blocks of the pair
                    i = 2 * P + e            # query block index (0..15)
                    ctx = ctx_pool.tile([128, 1024], F32)
                    sums = qtps_pool.tile([128, 2], F32, tag="qtps")
                    for j in range(3):       # key blocks of the band
                        lhsT = pt[:, e + j, e * 128 : (e + 1) * 128]
                        hb = i + j - rb0     # haug block within chunk
                        nc.tensor.matmul(
                            ctx[:, 0:512], lhsT, ha[:, hb, 0:512],
                            start=(j == 0), stop=(j == 2),
                        )
                        nc.tensor.matmul(
                            ctx[:, 512:1024], lhsT, ha[:, hb, 512:1024],
                            start=(j == 0), stop=(j == 2),
                        )
                        nc.tensor.matmul(
                            sums[:], lhsT, ones_sb[:],
                            start=(j == 0), stop=(j == 2),
                        )
                    recip = recip_pool.tile([128, 1], F32)
                    nc.vector.reciprocal(recip[:], sums[:, 0:1])
                    ctxs = ctxs_pool.tile([128, 1024], F32)
                    nc.scalar.mul(ctxs[:], ctx[:], mul=recip[:])
                    nc.sync.dma_start(out[i], ctxs[:])


def _build():
    if "nc" in _CACHE:
        return _CACHE["nc"]
    nc = bacc.Bacc(
        "TRN2", target_bir_lowering=False, debug=False, num_devices=N_CORES
    )
    hTr = nc.dram_tensor("hTr", [128, 8, RB * 128], F32R, kind="ExternalInput").ap()
    haug = nc.dram_tensor("haug", [RB, 128, H], F32R, kind="ExternalInput").ap()
    Wr = nc.dram_tensor("Wr", [128, 8, H], F32R, kind="ExternalInput").ap()
    cmask = nc.dram_tensor("cmask", [128, 4, 256], F32, kind="ExternalInput").ap()
    bmask = nc.dram_tensor("bmask", [128, 2, 256], F32, kind="ExternalInput").ap()
    onesd = nc.dram_tensor("onesd", [128, 2], F32R, kind="ExternalInput").ap()
    out = nc.dram_tensor("out", [QB, 128, H], F32, kind="ExternalOutput").ap()
    with tile.TileContext(nc) as tc:
        _kernel_body(tc, out, hTr, haug, Wr, cmask, bmask, onesd)
    nc.compile()
    _CACHE["nc"] = nc
    return nc


def _host_masks():
    kk = np.arange(128, dtype=np.int64)[:, None]
    qi = np.arange(128, dtype=np.int64)[None, :]
    su = np.where(kk > qi, np.float32(0.0), NEG).astype(np.float32)
    caus = np.where(kk <= qi, np.float32(0.0), NEG).astype(np.float32)
    full = np.zeros((128, 128), np.float32)
    inv = np.full((128, 128), NEG, np.float32)
    cmask = np.empty((128, 4, 256), np.float32)
    cmask[:, 0, 0:128], cmask[:, 0, 128:] = su, inv
    cmask[:, 1, 0:128], cmask[:, 1, 128:] = full, su
    cmask[:, 2, 0:128], cmask[:, 2, 128:] = caus, full
    cmask[:, 3, 0:128], cmask[:, 3, 128:] = inv, caus
    bmask_boundary = np.full((128, 2, 256), NEG, np.float32)
    bmask_interior = np.ascontiguousarray(cmask[:, 0:2, :])
    return cmask, bmask_boundary, bmask_interior


def _prepare_in_maps(h, W):
    cmask, bmask_b, bmask_i = _host_masks()
    in_maps = []
    for core in range(N_CORES):
        b, half = core // 2, core % 2
        k_lo = half * QPC - 256            # region global key start
        pad = max(0, -k_lo)                # 256 for half 0, else 0
        k_lo = max(0, k_lo)
        k_hi = half * QPC + QPC
        n_real = k_hi - k_lo

        # feature-major region [128, 8, 2304]
        hT_region = np.zeros((H, RB * 128), np.float32)
        hT_region[:, pad:] = h[b, k_lo:k_hi].T
        hTr = np.ascontiguousarray(
            hT_region.reshape(8, 128, RB * 128).transpose(1, 0, 2)
        )

        # keys-major region [18, 128, 1024]
        haug = np.zeros((RB, 128, H), np.float32)
        haug.reshape(RB * 128, H)[pad:] = h[b, k_lo:k_hi]

        in_maps.append(
            {
                "hTr": hTr,
                "haug": haug,
                "Wr": np.ascontiguousarray(
                    W.reshape(8, 128, H).transpose(1, 0, 2)
                ),
                "cmask": cmask,
                "bmask": bmask_b if half == 0 else bmask_i,
                "onesd": np.ones((128, 2), np.float32),
            }
        )
    return in_maps


def _assemble(results):
    out = np.empty((B, T, H), np.float32)
    for core in range(N_CORES):
        b, half = core // 2, core % 2
        out[b, half * QPC : (half + 1) * QPC] = (
            results[core]["out"].reshape(QPC, H)
        )
    return out


def kernel(h, W, T_hist):
    h = np.asarray(h, dtype=np.float32)
    W = np.asarray(W, dtype=np.float32)
    assert int(T_hist) == T_HIST
    assert h.shape == (B, T, H) and W.shape == (H, H)

    from concourse.bass_utils import run_bass_kernel_spmd

    nc = _build()
    in_maps = _prepare_in_maps(h, W)
    res = run_bass_kernel_spmd(nc, in_maps, core_ids=list(range(N_CORES)))
    return _assemble(res.results)


# revision 6
# speedup vs baseline: 1.1503x; 1.1503x over previous
"""Banded (Luong) attention TRN2 Bass kernel, 8-core SPMD.

Problem: h [4, 4096, 1024] f32, W [1024, 1024] f32, T_hist=256.
  K = h @ W.T ; scores = (h @ K^T) / sqrt(H) with causal band
  (q - 255 <= k <= q); out = softmax(scores) @ h.

Sharding: data-parallel over batch (4) x sequence halves (2) -> 8 cores,
no cross-core communication. Each core handles 2048 queries; its key
region is 18 blocks of 128 (2 lead blocks for the band history,
zero-padded for the first half of each sequence).

Per-core algorithm (all matmuls in float32r = fp32 with 11-bit mantissa,
full PE speed):
  Q' = h_q @ W            (query-side projection; scores = Q' @ h_k^T)
  ST[k, q] = h_k @ Q'^T   computed per 256-query pair over 4 key blocks,
                          masked additively, exp with fused 1/32 scale
  ctx[q, :] = PT^T @ h    accumulated over 3 key blocks; row sums via a
                          ones-column matmul; normalize during the
                          PSUM->SBUF copy with a per-partition 1/sum.
"""

import numpy as np

import concourse.bass as bass
import concourse.mybir as mybir
import concourse.tile as tile
from concourse import bacc

B, T, H = 4, 4096, 1024
T_HIST = 256
N_CORES = 8
QB = 16            # 128-row query blocks per core
RB = QB + 2        # key-region blocks per core (2 lead blocks)
QPC = 2048         # queries per core
F32R = mybir.dt.float32r
F32 = mybir.dt.float32
NEG = np.float32(-1e9)
INV_SQRT_H = 1.0 / 32.0

_CACHE = {}


def _kernel_body(tc, out, hTr, haug, Wr, cmask, bmask, onesd):
    nc = tc.nc

    with (
        tc.tile_pool(name="singles", bufs=1) as singles,
        tc.tile_pool(name="haug", bufs=1) as haug_pool,
        tc.tile_pool(name="qt", bufs=1) as qt_pool,
        tc.tile_pool(name="pt", bufs=2) as pt_pool,
        tc.tile_pool(name="ctxs", bufs=3) as ctxs_pool,
        tc.tile_pool(name="recip", bufs=4) as recip_pool,
        tc.tile_pool(name="qtps", bufs=2, space="PSUM") as qtps_pool,
        tc.tile_pool(name="st", bufs=2, space="PSUM") as st_pool,
        tc.tile_pool(name="ctx", bufs=1, space="PSUM") as ctx_pool,
    ):
        # --- resident inputs, DMA'd in consumption order so the first
        # projection group starts after ~3 MiB instead of ~15 MiB ---
        cm_sb = singles.tile([128, 4, 256], F32)
        nc.sync.dma_start(cm_sb[:], cmask[:])
        bm_sb = singles.tile([128, 2, 256], F32)
        nc.sync.dma_start(bm_sb[:], bmask[:])
        ones_sb = singles.tile([128, 2], F32R)
        nc.sync.dma_start(ones_sb[:], onesd[:])

        hT_sb = singles.tile([128, 8, RB * 128], F32R)   # 9.44 MiB
        W_sb = singles.tile([128, 8, H], F32R)           # 4 MiB
        nc.sync.dma_start(W_sb[:, :, 0:128], Wr[:, :, 0:128])
        nc.sync.dma_start(hT_sb[:, :, 256:768], hTr[:, :, 256:768])
        nc.sync.dma_start(W_sb[:, :, 128:1024], Wr[:, :, 128:1024])
        nc.sync.dma_start(hT_sb[:, :, 768:1280], hTr[:, :, 768:1280])
        nc.sync.dma_start(hT_sb[:, :, 0:256], hTr[:, :, 0:256])

        ha0 = haug_pool.tile([128, 10, H], F32R, tag="ha")  # 5.24 MiB
        nc.sync.dma_start(ha0[:], haug[0:10].rearrange("b p f -> p b f"))
        nc.sync.dma_start(hT_sb[:, :, 1280:2304], hTr[:, :, 1280:2304])

        for c in range(2):  # two chunks of 8 query blocks
            rb0 = c * 8                      # first region block of chunk
            # values for this chunk: region blocks rb0..rb0+9, keys-major
            if c == 0:
                ha = ha0
            else:
                ha = haug_pool.tile([128, 10, H], F32R, tag="ha")
                nc.sync.dma_start(
                    ha[:], haug[rb0 : rb0 + 10].rearrange("b p f -> p b f")
                )

            # --- projection: Q'T[m, q] for this chunk's 1024 queries ---
            qt = qt_pool.tile([128, 8, 1024], F32R)      # 4 MiB
            qcol0 = 256 + c * 1024           # region col of chunk's queries
            for tt in range(2):
                for mc in range(8):
                    ps = qtps_pool.tile([128, 512], F32, tag="qtps")
                    for oc in range(8):
                        nc.tensor.matmul(
                            ps[:],
                            W_sb[:, oc, mc * 128 : (mc + 1) * 128],
                            hT_sb[:, oc, qcol0 + tt * 512 : qcol0 + (tt + 1) * 512],
                            start=(oc == 0),
                            stop=(oc == 7),
                        )
                    nc.vector.tensor_copy(
                        qt[:, mc, tt * 512 : (tt + 1) * 512], ps[:]
                    )

            # --- attention: 4 pairs (of 2 query blocks) per chunk ---
            for p in range(4):
                P = c * 4 + p                # global pair index
                st = st_pool.tile([128, 4, 256], F32)
                for r in range(4):
                    for mc in range(8):
                        nc.tensor.matmul(
                            st[:, r, :],
                            hT_sb[:, mc, (2 * P + r) * 128 : (2 * P + r + 1) * 128],
                            qt[:, mc, p * 256 : (p + 1) * 256],
                            start=(mc == 0),
                            stop=(mc == 7),
                        )
                if P == 0:
                    nc.vector.tensor_add(st[:, 0:2, :], st[:, 0:2, :], bm_sb[:])
                    nc.vector.tensor_add(
                        st[:, 2:4, :], st[:, 2:4, :], cm_sb[:, 2:4, :]
                    )
                else:
                    nc.vector.tensor_add(st[:], st[:], cm_sb[:])
                pt = pt_pool.tile([128, 4, 256], F32R)
                nc.scalar.activation(
                    pt[:], st[:], mybir.ActivationFunctionType.Exp,
                    scale=INV_SQRT_H,
                )

                for e in range(2):           # the two query blocks of the pair
                    i = 2 * P + e            # query block index (0..15)
                    ctx = ctx_pool.tile([128, 1024], F32)
                    sums = qtps_pool.tile([128, 2], F32, tag="qtps")
                    for j in range(3):       # key blocks of the band
                        lhsT = pt[:, e + j, e * 128 : (e + 1) * 128]
                        hb = i + j - rb0     # haug block within chunk
                        nc.tensor.matmul(
                            ctx[:, 0:512], lhsT, ha[:, hb, 0:512],
                            start=(j == 0), stop=(j == 2),
                        )
                        nc.tensor.matmul(
                            ctx[:, 512:1024], lhsT, ha[:, hb, 512:1024],
                            start=(j == 0), stop=(j == 2),
                        )
                        nc.tensor.matmul(
                            sums[:], lhsT, ones_sb[:],
                            start=(j == 0), stop=(j == 2),
                        )
                    recip = recip_pool.tile([128, 1], F32)
                    nc.vector.reciprocal(recip[:], sums[:, 0:1])
                    ctxs = ctxs_pool.tile([128, 1024], F32)
                    nc.scalar.mul(ctxs[:], ctx[:], mul=recip[:])
                    nc.sync.dma_start(out[i], ctxs[:])


def _build():
    if "nc" in _CACHE:
        return _CACHE["nc"]
    nc = bacc.Bacc(
        "TRN2", target_bir_lowering=False, debug=False, num_devices=N_CORES
    )
    hTr = nc.dram_tensor("hTr", [128, 8, RB * 128], F32R, kind="ExternalInput").ap()
    haug = nc.dram_tensor("haug", [RB, 128, H], F32R, kind="ExternalInput").ap()
    Wr = nc.dram_tensor("Wr", [128, 8, H], F32R, kind="ExternalInput").ap()
    cmask = nc.dram_tensor("cmask", [128, 4, 256], F32, kind="ExternalInput").ap()
    bmask = nc.dram_tensor("bmask", [128, 2, 256], F32, kind="ExternalInput").ap()
    onesd = nc.dram_tensor("onesd", [128, 2], F32R, kind="ExternalInput").ap()
    out = nc.dram_tensor("out", [QB, 128, H], F32, kind="ExternalOutput").ap()
    with tile.TileContext(nc) as tc:
        _kernel_body(tc, out, hTr, haug, Wr, cmask, bmask, onesd)
    nc.compile()
    _CACHE["nc"] = nc
    return nc


def _host_masks():
    kk = np.arange(128, dtype=np.int64)[:, None]
    qi = np.arange(128, dtype=np.int64)[None, :]
    su = np.where(kk > qi, np.float32(0.0), NEG).astype(np.float32)
    caus = np.where(kk <= qi, np.float32(0.0), NEG).astype(np.float32)
    full = np.zeros((128, 128), np.float32)
    inv = np.full((128, 128), NEG, np.float32)
    cmask = np.empty((128, 4, 256), np.float32)
    cmask[:, 0, 0:128], cmask[:, 0, 128:] = su, inv
    cmask[:, 1, 0:128], cmask[:, 1, 128:] = full, su
    cmask[:, 2, 0:128], cmask[:, 2, 128:] = caus, full
    cmask[:, 3, 0:128], cmask[:, 3, 128:] = inv, caus
    bmask_boundary = np.full((128, 2, 256), NEG, np.float32)
    bmask_interior = np.ascontiguousarray(cmask[:, 0:2, :])
    return cmask, bmask_boundary, bmask_interior


def _prepare_in_maps(h, W):
    cmask, bmask_b, bmask_i = _host_masks()
    in_maps = []
    for core in range(N_CORES):
        b, half = core // 2, core % 2
        k_lo = half * QPC - 256            # region global key start
        pad = max(0, -k_lo)                # 256 for half 0, else 0
        k_lo = max(0, k_lo)
        k_hi = half * QPC + QPC
        n_real = k_hi - k_lo

        # feature-major region [128, 8, 2304]
        hT_region = np.zeros((H, RB * 128), np.float32)
        hT_region[:, pad:] = h[b, k_lo:k_hi].T
        hTr = np.ascontiguousarray(
            hT_region.reshape(8, 128, RB * 128).transpose(1, 0, 2)
        )

        # keys-major region [18, 128, 1024]
        haug = np.zeros((RB, 128, H), np.float32)
        haug.reshape(RB * 128, H)[pad:] = h[b, k_lo:k_hi]

        in_maps.append(
            {
                "hTr": hTr,
                "haug": haug,
                "Wr": np.ascontiguousarray(
                    W.reshape(8, 128, H).transpose(1, 0, 2)
                ),
                "cmask": cmask,
                "bmask": bmask_b if half == 0 else bmask_i,
                "onesd": np.ones((128, 2), np.float32),
            }
        )
    return in_maps


def _assemble(results):
    out = np.empty((B, T, H), np.float32)
    for core in range(N_CORES):
        b, half = core // 2, core % 2
        out[b, half * QPC : (half + 1) * QPC] = (
            results[core]["out"].reshape(QPC, H)
        )
    return out


def kernel(h, W, T_hist):
    h = np.asarray(h, dtype=np.float32)
    W = np.asarray(W, dtype=np.float32)
    assert int(T_hist) == T_HIST
    assert h.shape == (B, T, H) and W.shape == (H, H)

    from concourse.bass_utils import run_bass_kernel_spmd

    nc = _build()
    in_maps = _prepare_in_maps(h, W)
    res = run_bass_kernel_spmd(nc, in_maps, core_ids=list(range(N_CORES)))
    return _assemble(res.results)
